# revision 63
# baseline (speedup 1.0000x reference)
"""AttentionBlock (GroupNorm -> 1x1-conv QKV -> 4-head attention -> 1x1-conv proj
-> residual) on 8 Trainium2 NeuronCores.

Sharding: pure data-parallel over batch (16 batches -> 2 per core). Each core
runs an identical Bass/Tile program on its 2 batches; no collectives.

v2: fp8e4 DoubleRow matmuls on every K>=256 contraction. DoubleRow packs two
128-deep k-subtiles into one PE pass (2 fp8 rhs rows/cycle), halving matmul
count for qkv, P@V, colsum and proj vs f32r. ST (logits) keeps f32r: its
contraction is head_dim=128, which DoubleRow cannot pair without summing
heads. Weights are host-quantized to fp8 with a x16 scale (keeps the
~N(0,1/512) entries out of the e4m3 subnormal floor); the 1/16 descale folds
into the PSUM evacuations. PT=exp(s)/4 is stored fp8 (max logit 6.4 ->
exp/4 = 148 < e4m3 max 240); the /4 cancels between P@V and the colsum.

Softmax normalization is per-head-pipelined: each head's colsum goes to its
own ping-pong PSUM region ([2,512]: ch halves on partitions 0/1 via one-hot
DoubleRow lhsT), so r_h = 16/colsum (ln->exp, the x16 preps fp8 o) and the
o_h = O_h * r_h normalize run while later heads are still in ST/PV. proj
(DoubleRow over head pairs) + residual is a single scalar_tensor_tensor
(x + psum/256) per channel tile, split DVE/Pool.

GroupNorm: unchanged from v1 (bn-stats via DVE reduce + ACT Square accum,
selector matmuls for the group reduce/broadcast), but xn is written fp8.
"""

import math

import numpy as np

B, CH, HW = 16, 512, 1024           # full problem: x [16, 512, 32, 32]
NCORES = 8
BLOC = B // NCORES                  # batches per core
NH = 4                              # heads
HD = 128                            # head dim
GROUPS = 32
GSIZE = CH // GROUPS                # 16 channels per group
EPS = 1e-5
CT = CH // 128                      # channel tiles = 4
NT = HW // 128                      # n tiles = 8
NTP = NT // 2                       # nt pairs (DoubleRow)
SCALE = 1.0 / float(np.sqrt(HD))
PT_BIAS = float(-2.0 * math.log(2.0))   # pt = exp(s)/4, keeps pt < 240
R_BIAS = float(math.log(16.0))          # rb = 16/colsum -> o_pair = 16*o_norm
WSCALE = 16.0                       # host weight multiplier before fp8 cast

TAIL_RBP = True                     # PE-broadcast r for the tail head
TAIL_IMM = True                     # identity-matmul residual for batch 1
MID_GN = True                       # emit gn(1) inside attention(0)
GN1_DVE = True                      # gn(1) sum-of-squares via DVE
PEND_DEPTH = 3                      # attention software pipeline depth
TRACE = False                       # set by the test harness for NTFF profiling
LAST = {}                           # exec_time_ns etc. from the last traced run

_cache = {}


def _consts():
    """Host-side constant matrices fed as DRAM inputs (shared by all cores)."""
    import ml_dtypes

    f8 = ml_dtypes.float8_e4m3
    sel16 = np.zeros((128, CT, GROUPS), np.float32)   # group-average selector
    selT = np.zeros((GROUPS, CT, 128), np.float32)    # group -> channel bcast
    for t in range(CT):
        for p in range(128):
            g = 8 * t + p // GSIZE
            sel16[p, t, g] = 1.0 / (GSIZE * HW)   # raw sums -> mean, E[x^2]
            selT[g, t, p] = 1.0
    # colsum one-hot lhsT: csw[:, ch] is [128, 2(sub), 16(M)] all-ones in
    # column ch -> out [16, 512] row ch = column sums over both subtiles.
    # M=16 because dual-fp8 ldweights rejects smaller weight tiles
    # (s3_lw_dual_fp8_restrictions).
    csw = np.zeros((128, 2, 2, 16), np.float32)
    csw[:, 0, :, 0] = 1.0
    csw[:, 1, :, 1] = 1.0
    return dict(
        sel16=sel16.reshape(128, CT * GROUPS),
        selT=selT.reshape(GROUPS, CT * 128),
        csw=csw.reshape(128, 64).astype(f8),
        ones128=np.ones((1, 128), np.float32).astype(f8),
        onesf=np.ones((1, 128), np.float32),
        id256=np.eye(128, dtype=np.float32) * (WSCALE * WSCALE),
    )


def _pin_act_tables():
    """Make exp/ln resolvable only via 'natural_log_exp_and_others' so the
    whole kernel uses a single activation table set (indices preserved)."""
    import functools

    import concourse.bacc as bacc_mod
    from concourse import hw_specs, mybir

    if getattr(hw_specs.get_activation_tables, "_pinned", False):
        return
    orig = hw_specs.get_activation_tables

    @functools.cache
    def pinned(arch):
        t = dict(orig(arch))
        comb = "natural_log_exp_and_others"
        if comb in t:
            drop = {mybir.ActivationFunctionType.Exp,
                    mybir.ActivationFunctionType.Ln,
                    mybir.ActivationFunctionType.Square,
                    mybir.ActivationFunctionType.Identity}
            for name in list(t):
                if name != comb:
                    t[name] = t[name] - drop
        return t

    pinned._pinned = True
    hw_specs.get_activation_tables = pinned
    bacc_mod.get_activation_tables = pinned


def _build(has_vbias=True, has_pbias=True):
    """Build the (finalized) Bacc graph for one core's 2-batch program."""
    import concourse.tile as tile
    from concourse import bacc, mybir

    _pin_act_tables()

    f32 = mybir.dt.float32
    f32r = mybir.dt.float32r
    fp8 = mybir.dt.float8e4
    DR = mybir.MatmulPerfMode.DoubleRow
    Alu = mybir.AluOpType
    Act = mybir.ActivationFunctionType

    nc = bacc.Bacc("TRN2", target_bir_lowering=False, debug=False,
                   num_devices=NCORES)

    # ---- DRAM I/O -----------------------------------------------------------
    x_d = nc.dram_tensor("x", [BLOC, CH, HW], f32, kind="ExternalInput")
    wqkvT_d = nc.dram_tensor("wqkvT", [CH, 3 * CH], fp8, kind="ExternalInput")
    wprojT_d = nc.dram_tensor("wprojT", [CH, CH], fp8, kind="ExternalInput")
    gnw_d = nc.dram_tensor("gnw", [128, CT], f32, kind="ExternalInput")
    gnb_d = nc.dram_tensor("gnb", [128, CT], f32, kind="ExternalInput")
    qbqk_d = nc.dram_tensor("qbqk", [128, 2 * CT], f32, kind="ExternalInput")
    qbv_d = nc.dram_tensor("qbv", [1, CH], fp8, kind="ExternalInput")
    pbcol_d = nc.dram_tensor("pbcol", [128, CT], f32, kind="ExternalInput")
    sel16_d = nc.dram_tensor("sel16", [128, CT * GROUPS], f32, kind="ExternalInput")
    selT_d = nc.dram_tensor("selT", [GROUPS, CT * 128], f32, kind="ExternalInput")
    csw_d = nc.dram_tensor("csw", [128, 64], fp8, kind="ExternalInput")
    ones128_d = nc.dram_tensor("ones128", [1, 128], fp8, kind="ExternalInput")
    onesf_d = nc.dram_tensor("onesf", [1, 128], f32r, kind="ExternalInput")
    id256_d = nc.dram_tensor("id256", [128, 128], f32r, kind="ExternalInput")
    out_d = nc.dram_tensor("out", [BLOC, CH, HW], f32, kind="ExternalOutput")
    rtd = nc.dram_tensor("rtd_scratch", [BLOC, NH, HW], f32)

    with tile.TileContext(nc) as tc:
        with (
            tc.tile_pool(name="wp", bufs=1) as wp,
            tc.tile_pool(name="dp", bufs=1) as dp,
            tc.tile_pool(name="gp", bufs=3) as gp,
            tc.tile_pool(name="ps", bufs=2, space="PSUM") as ps,
        ):
            # ---- DMAs: x first (GN can start), then qkv weights, then rest --
            x_sbs = []

            def load_x(b, engs):
                # half-tile DMAs so GN's per-half stats start sooner
                x_sb = dp.tile([128, CT, HW], f32, tag="x", bufs=2,
                               name=f"x_{b}")
                x_sbs.append(x_sb)
                for t in range(CT):
                    for hf in range(2):
                        engs[(2 * t + hf) % len(engs)].dma_start(
                            out=x_sb[:, t, hf * 512:(hf + 1) * 512],
                            in_=x_d[b, t * 128:(t + 1) * 128,
                                    hf * 512:(hf + 1) * 512])

            # warmup matmuls start as soon as wrm is set; memset on the
            # otherwise-idle gpsimd queue, first instruction of the kernel
            # (fp8: full-rate rows, no f32r-rounding verifier rules)
            wrm = wp.tile([128, 512], fp8)
            nc.gpsimd.memset(wrm, 0.00390625)

            load_x(0, [nc.sync, nc.gpsimd])

            sel16 = wp.tile([128, CT, GROUPS], f32)
            nc.sync.dma_start(out=sel16, in_=sel16_d[:, :].rearrange(
                "p (t g) -> p t g", t=CT))
            selT = wp.tile([GROUPS, CT, 128], f32)
            nc.sync.dma_start(out=selT, in_=selT_d[:, :].rearrange(
                "p (t g) -> p t g", t=CT))
            gnw = wp.tile([128, CT], f32)
            nc.sync.dma_start(out=gnw, in_=gnw_d[:, :])
            gnb = wp.tile([128, CT], f32)
            nc.sync.dma_start(out=gnb, in_=gnb_d[:, :])
            qbqk = wp.tile([128, 2 * CT], f32)
            nc.sync.dma_start(out=qbqk, in_=qbqk_d[:, :])
            qbv = wp.tile([1, CH], fp8)
            nc.sync.dma_start(out=qbv, in_=qbv_d[:, :])
            ones128 = wp.tile([1, 128], fp8)
            nc.sync.dma_start(out=ones128, in_=ones128_d[:, :])
            csw = wp.tile([128, 2, 2, 16], fp8)
            nc.sync.dma_start(out=csw, in_=csw_d[:, :].rearrange(
                "p (c s m) -> p c s m", c=2, s=2))
            epsc = wp.tile([128, 1], f32)
            nc.vector.memset(epsc, EPS)
            ptbc = wp.tile([128, 1], f32)
            nc.vector.memset(ptbc, PT_BIAS)
            rbbc = wp.tile([2, 1], f32)
            nc.vector.memset(rbbc, R_BIAS)
            pbcol = wp.tile([128, CT], f32)
            nc.sync.dma_start(out=pbcol, in_=pbcol_d[:, :])

            onesf = wp.tile([1, 128], f32r)
            nc.sync.dma_start(out=onesf, in_=onesf_d[:, :])
            id256 = wp.tile([128, 128], f32r)
            nc.sync.dma_start(out=id256, in_=id256_d[:, :])

            # w_qkv rides the sync queue AFTER the consts so its transfers
            # don't steal HBM bandwidth from the GN-critical x(0) tiles on
            # gpsimd (it is not needed until qkv(0), ~6us after GN ends)
            w_qkv = wp.tile([128, CT, 3 * CH], fp8)
            for k in range(CT):
                nc.sync.dma_start(out=w_qkv[:, k, :],
                                  in_=wqkvT_d[k * 128:(k + 1) * 128, :])

            # x(1) is only needed once gn_stats(1) runs, mid-attention(0)
            load_x(1, [nc.sync, nc.gpsimd])
            w_proj = wp.tile([128, CT, CH], fp8)
            for k in range(CT):
                nc.sync.dma_start(out=w_proj[:, k, :],
                                  in_=wprojT_d[k * 128:(k + 1) * 128, :])
            # pre-rounded f32r copy of batch 1's x for the tail residual
            # matmul (the BIR verifier requires f32r-matmul operands to be
            # PRODUCED as f32r; a bitcast of the f32 x tile is rejected).
            xr_sb = None
            if TAIL_IMM and not has_pbias:
                xr_sb = dp.tile([128, CT, HW], f32r, tag="xr", bufs=1,
                                name="xr_1")
                for t in range(CT):
                    eng = nc.sync if t % 2 == 0 else nc.gpsimd
                    eng.dma_start(
                        out=xr_sb[:, t, :],
                        in_=x_d[BLOC - 1,
                                t * 128:(t + 1) * 128, :].bitcast(f32r))

            def warmup(tag, n):
                # Throwaway matmuls that keep the PE activity monitor in the
                # full-clock state across otherwise-idle windows (results are
                # never read). WAW on one psum slot serializes them. f32r
                # bitcast: full-rate rows, 4x cheaper than plain f32.
                wps = ps.tile([128, 1024], f32, tag="st", name=f"warm_{tag}")
                for i in range(n):
                    nc.tensor.matmul(wps[:128, 0:512], lhsT=wrm[:, 0:128],
                                     rhs=wrm[:, :], start=True, stop=True)

            # ---------------- phase builders --------------------------------
            def gn_stats(b, on_dve=False, ts=None, store=None):
                # raw sums per channel: col0 = sum(x), col1 = sum(x^2). The
                # selector matmul carries the 1/(16*1024) factor.
                # b=0 splits across DVE reduce + ACT Square-accum (both idle
                # at startup); b=1 runs entirely on DVE (square via plain
                # tensor_tensor then reduce) because its stats are emitted
                # mid-attention(0), where ACT is saturated with exps.
                # ts/store allow emitting a subset of tiles per call so the
                # DVE work spreads across attention(0)'s per-head slack.
                x_sb = x_sbs[b]
                if store is None:
                    store = {}
                if "xn" not in store:
                    store["xn"] = dp.tile([128, CT, HW], fp8, tag="xn",
                                          bufs=2, name=f"xn_{b}")
                    store["sq"] = dp.tile([128, HW], f32, tag="sq", bufs=1,
                                          name="sq_scratch")
                    store["pks"] = []
                xn_sb, sq_sb, pks = store["xn"], store["sq"], store["pks"]
                for t in (range(CT) if ts is None else ts):
                    # per-half stats (cols: sum_h0, sq_h0, sum_h1, sq_h1) so
                    # each piece starts as soon as its half-tile DMA lands;
                    # gn_finish folds halves with one [32,2]+[32,2] add
                    pk = gp.tile([128, 4], f32, tag="pk", bufs=9,
                                 name=f"pk_{b}_{t}")
                    for hf in range(2):
                        xh = x_sb[:, t, hf * 512:(hf + 1) * 512]
                        nc.vector.tensor_reduce(
                            out=pk[:, 2 * hf:2 * hf + 1], in_=xh,
                            axis=mybir.AxisListType.X, op=Alu.add)
                        if on_dve:
                            nc.vector.tensor_tensor(out=sq_sb[:, 0:512],
                                                    in0=xh, in1=xh,
                                                    op=Alu.mult)
                            nc.vector.tensor_reduce(
                                out=pk[:, 2 * hf + 1:2 * hf + 2],
                                in_=sq_sb[:, 0:512],
                                axis=mybir.AxisListType.X, op=Alu.add)
                        else:
                            nc.scalar.activation(
                                out=sq_sb[:, 0:512], in_=xh,
                                func=Act.Square,
                                accum_out=pk[:, 2 * hf + 1:2 * hf + 2])
                    pks.append(pk)
                return xn_sb, pks

            def gn_finish(b, xn_sb, pks):
                x_sb = x_sbs[b]
                gstat = ps.tile([128, 1024], f32, tag="st", name=f"gstat_{b}")
                for t in range(CT):
                    nc.tensor.matmul(gstat[:GROUPS, 0:4], lhsT=sel16[:, t, :],
                                     rhs=pks[t][:, :],
                                     start=(t == 0), stop=(t == CT - 1))

                # fold the per-half stats: gs = [mean, E[x^2]] per group
                gs4 = gp.tile([32, 4], f32, tag="gs4", name=f"gs4_{b}")
                nc.vector.tensor_copy(out=gs4, in_=gstat[:GROUPS, 0:4])
                gs = gp.tile([32, 2], f32, tag="gs", name=f"gs_{b}")
                nc.vector.tensor_tensor(out=gs, in0=gs4[:, 0:2],
                                        in1=gs4[:, 2:4], op=Alu.add)
                m2 = gp.tile([32, 1], f32, tag="m2", name=f"m2_{b}")
                nc.vector.tensor_scalar(out=m2, in0=gs[:, 0:1],
                                        scalar1=gs[:, 0:1], scalar2=None,
                                        op0=Alu.mult)
                varv = gp.tile([32, 1], f32, tag="varv", name=f"varv_{b}")
                nc.vector.tensor_tensor(out=varv, in0=gs[:, 1:2], in1=m2,
                                        op=Alu.subtract)
                lnv = gp.tile([32, 1], f32, tag="lnv", name=f"lnv_{b}")
                nc.scalar.activation(out=lnv, in_=varv, func=Act.Ln,
                                     bias=epsc[:GROUPS, :])
                st2 = gp.tile([32, 2], f32, tag="st2", name=f"st2_{b}")
                nc.scalar.activation(out=st2[:, 1:2], in_=lnv, func=Act.Exp,
                                     scale=-0.5)
                nc.vector.tensor_copy(out=st2[:, 0:1], in_=gs[:, 0:1])

                # all four broadcast matmuls up front into ONE psum tile
                # (disjoint column pairs), then the per-tile DVE chains:
                # grouped emission keeps the later tiles' matmuls from being
                # scheduled ~6us late behind unrelated DVE work.
                cst = ps.tile([128, 1024], f32, tag="st", name=f"cst_{b}")
                for t in range(CT):
                    nc.tensor.matmul(cst[:, 2 * t:2 * t + 2],
                                     lhsT=selT[:, t, :],
                                     rhs=st2[:, :], start=True, stop=True)
                for t in range(CT):
                    c0 = cst[:, 2 * t:2 * t + 1]
                    c1 = cst[:, 2 * t + 1:2 * t + 2]
                    ab = gp.tile([128, 2], f32, tag="ab", bufs=5,
                                 name=f"ab_{b}_{t}")
                    nc.vector.tensor_tensor(out=ab[:, 0:1], in0=c1,
                                            in1=gnw[:, t:t + 1], op=Alu.mult)
                    t1 = gp.tile([128, 1], f32, tag="t1", name=f"t1_{b}_{t}")
                    nc.vector.tensor_tensor(out=t1, in0=c0,
                                            in1=ab[:, 0:1], op=Alu.mult)
                    nc.vector.tensor_tensor(out=ab[:, 1:2], in0=gnb[:, t:t + 1],
                                            in1=t1, op=Alu.subtract)
                    nc.vector.tensor_scalar(
                        out=xn_sb[:, t, :], in0=x_sb[:, t, :],
                        scalar1=ab[:, 0:1], scalar2=ab[:, 1:2],
                        op0=Alu.mult, op1=Alu.add)
                    if has_pbias:
                        # fold proj bias into the residual base (x += proj_b)
                        nc.vector.tensor_scalar(
                            out=x_sb[:, t, :], in0=x_sb[:, t, :],
                            scalar1=pbcol[:, t:t + 1], scalar2=None,
                            op0=Alu.add)
                return xn_sb

            def qkv(b, xn_sb):
                # All matmuls are emitted kp0-first within psum-tile pairs so
                # the first k-pair's work (only needs xn tiles 0,1) streams
                # while xn tiles 2,3 are still being produced at startup.
                q_sb = dp.tile([128, NH, HW], f32r, tag="q", bufs=1,
                               name=f"q_{b}")
                k_sb = dp.tile([128, NH, HW], f32r, tag="k", bufs=1,
                               name=f"k_{b}")
                vT_sb = dp.tile([128, NT, 512], fp8, tag="vT", bufs=1,
                                name=f"vT_{b}")

                def qk_mm(ps_t, col0, kp):
                    for ch in range(2):
                        nc.tensor.matmul(
                            ps_t[:, ch * 512:(ch + 1) * 512],
                            lhsT=w_qkv[:, 2 * kp:2 * kp + 2,
                                       col0:col0 + 128],
                            rhs=xn_sb[:, 2 * kp:2 * kp + 2,
                                      ch * 512:(ch + 1) * 512],
                            start=(kp == 0), stop=(kp == 1),
                            perf_mode=DR)

                def q_evac(mt, pq):
                    nc.scalar.activation(out=q_sb[:, mt, :], in_=pq,
                                         func=Act.Identity,
                                         scale=1.0 / WSCALE,
                                         bias=qbqk[:, mt:mt + 1])

                def k_evac(mt, pk_):
                    nc.vector.tensor_scalar(
                        out=k_sb[:, mt, :], in0=pk_,
                        scalar1=1.0 / WSCALE,
                        scalar2=qbqk[:, NH + mt:NH + mt + 1],
                        op0=Alu.mult, op1=Alu.add)

                for qk in range(2):            # 0 = q, 1 = k
                    evac = q_evac if qk == 0 else k_evac
                    for mta in (0, 2):
                        pts = {}
                        for mt in (mta, mta + 1):
                            pts[mt] = ps.tile([128, 1024], f32, tag="st",
                                              name=f"pqk_{b}_{qk}_{mt}")
                            qk_mm(pts[mt], qk * 512 + mt * 128, 0)
                        for mt in (mta, mta + 1):
                            qk_mm(pts[mt], qk * 512 + mt * 128, 1)
                            evac(mt, pts[mt])
                for nta in range(0, NT, 2):    # vT tiles, pairwise
                    pvs = {}
                    for nt in (nta, nta + 1):
                        pvs[nt] = ps.tile([128, 1024], f32, tag="st",
                                          name=f"pv_{b}_{nt}")
                        nc.tensor.matmul(
                            pvs[nt][:, 0:512],
                            lhsT=xn_sb[:, 0:2, nt * 128:(nt + 1) * 128],
                            rhs=w_qkv[:, 0:2, 1024:1536],
                            start=True, stop=False, perf_mode=DR)
                    for nt in (nta, nta + 1):
                        nc.tensor.matmul(
                            pvs[nt][:, 0:512],
                            lhsT=xn_sb[:, 2:4, nt * 128:(nt + 1) * 128],
                            rhs=w_qkv[:, 2:4, 1024:1536],
                            start=False, stop=not has_vbias, perf_mode=DR)
                        if has_vbias:
                            nc.tensor.matmul(pvs[nt][:, 0:512],
                                             lhsT=ones128[:, :],
                                             rhs=qbv[:, :],
                                             start=False, stop=True)
                        if nt % 2 == 0:
                            nc.scalar.activation(out=vT_sb[:, nt, :],
                                                 in_=pvs[nt][:, 0:512],
                                                 func=Act.Identity,
                                                 scale=1.0 / WSCALE)
                        else:
                            nc.vector.tensor_scalar(
                                out=vT_sb[:, nt, :], in0=pvs[nt][:, 0:512],
                                scalar1=1.0 / WSCALE, scalar2=None,
                                op0=Alu.mult)
                return q_sb, k_sb, vT_sb

            def attention(b, q_sb, k_sb, vT_sb, mid_cb=None):
                # Software-pipelined: ST/exp of step i+1 is emitted BEFORE
                # PV/cs of step i, so the PE always has independent matmuls
                # in its (in-order) queue while ACT computes exp(i).
                # Per-head finish: the head's colsum lives in its own
                # ping-pong PSUM region, so r_h and the o-normalize run
                # while later heads compute.
                ov = ps.tile([128, 1024], f32, tag="ov", bufs=1,
                             name=f"ov_{b}")
                o_pairs = [dp.tile([128, 2, HW], fp8, tag="op", bufs=2,
                                   name=f"op_{b}_{i}") for i in range(2)]
                # per-head ping-pong colsum regions; the tail head gets TWO
                # tiles (one per ch, both row 0) so its ln/exp can read at
                # partition 0 (engine reads cannot start mid-partition).
                cs_tiles = []
                for h in range(NH):
                    if TAIL_RBP and b == BLOC - 1 and h == NH - 1:
                        cs_tiles.append(tuple(
                            ps.tile([16, 512], f32, tag="cs", bufs=2,
                                    name=f"cs_{b}_{h}_{ch}")
                            for ch in range(2)))
                    else:
                        cs_tiles.append(ps.tile([16, 512], f32, tag="cs",
                                                bufs=2, name=f"cs_{b}_{h}"))

                def st_exp(h, tp):
                    ptp = dp.tile([128, 2, HW], fp8, tag="pt",
                                  bufs=PEND_DEPTH + 1,
                                  name=f"pt_{b}_{h}_{tp}")
                    for i in range(2):
                        nt = 2 * tp + i
                        stp = ps.tile([128, 1024], f32, tag="st",
                                      name=f"stp_{b}_{h}_{nt}")
                        for ch in range(2):
                            nc.tensor.matmul(
                                stp[:, ch * 512:(ch + 1) * 512],
                                lhsT=k_sb[:, h, nt * 128:(nt + 1) * 128],
                                rhs=q_sb[:, h, ch * 512:(ch + 1) * 512],
                                start=True, stop=True)
                        nc.scalar.activation(out=ptp[:, i, :], in_=stp,
                                             func=Act.Exp, scale=SCALE,
                                             bias=ptbc[:, :])
                    return ptp

                def pv_cs(h, tp, ptp):
                    # PV halves first: their stop releases the O evacuation
                    # (which gates the next head's first PV) two matmuls
                    # earlier; the colsum->r chain has a full head of slack.
                    csp = cs_tiles[h]
                    split = isinstance(csp, tuple)
                    for ch in range(2):
                        nc.tensor.matmul(
                            ov[:, ch * 512:(ch + 1) * 512],
                            lhsT=vT_sb[:, 2 * tp:2 * tp + 2,
                                       h * 128:(h + 1) * 128],
                            rhs=ptp[:, :, ch * 512:(ch + 1) * 512],
                            start=(tp == 0), stop=(tp == NTP - 1),
                            perf_mode=DR)
                    for ch in range(2):
                        nc.tensor.matmul(
                            (csp[ch] if split else csp)[0:16, 0:512],
                            lhsT=csw[:, 0 if split else ch],
                            rhs=ptp[:, :, ch * 512:(ch + 1) * 512],
                            start=(tp == 0 and (split or ch == 0)),
                            stop=(tp == NTP - 1),
                            perf_mode=DR)
                    if tp == NTP - 1:
                        finish_head(h, csp)

                def finish_head(h, csp):
                    # O evacuation (frees ov for the next head): two DVE
                    # halves so the next head's ch0 PV only waits on the
                    # first (GPSIMD cannot read PSUM; its tensor ops are
                    # also ~2.6x slower than DVE).
                    ost = dp.tile([128, HW], f32, tag="ost", bufs=2,
                                  name=f"ost_{b}_{h}")
                    nc.vector.tensor_copy(out=ost[:, 0:512],
                                          in_=ov[:, 0:512])
                    nc.vector.tensor_copy(out=ost[:, 512:1024],
                                          in_=ov[:, 512:1024])
                    op = o_pairs[h // 2]
                    if TAIL_RBP and b == BLOC - 1 and h == NH - 1:
                        # tail-critical head: broadcast r across partitions
                        # with K=1 PE matmuls into PSUM (the ST banks are
                        # free by now) instead of the ~1.3us DRAM bounce;
                        # also keeps the PE busy into proj. Per-ch [1,512]
                        # ln/exp tiles at partition 0 (matmul rhs must share
                        # the lhsT base partition).
                        rbp = ps.tile([128, 1024], f32, tag="st",
                                      name=f"rbp_{b}_{h}")
                        for ch in range(2):
                            sl = slice(ch * 512, (ch + 1) * 512)
                            ln1 = gp.tile([1, 512], f32, tag="ln1", bufs=2,
                                          name=f"ln1_{b}_{h}_{ch}")
                            nc.scalar.activation(out=ln1,
                                                 in_=csp[ch][0:1, 0:512],
                                                 func=Act.Ln)
                            rt1 = gp.tile([1, 512], f32r, tag="rt1", bufs=2,
                                          name=f"rt1_{b}_{h}_{ch}")
                            nc.scalar.activation(out=rt1, in_=ln1,
                                                 func=Act.Exp, scale=-1.0,
                                                 bias=rbbc[0:1, :])
                            nc.tensor.matmul(
                                rbp[:, sl], lhsT=onesf, rhs=rt1,
                                start=True, stop=True)
                            nc.vector.tensor_tensor(out=op[:, h % 2, sl],
                                                    in0=ost[:, sl],
                                                    in1=rbp[:, sl],
                                                    op=Alu.mult)
                        return
                    # r_h = 16/colsum via exp(-ln(cs)+ln16) on ACT (DVE's
                    # InstReciprocal measured 3.3us for [2,512] -- 5x the
                    # ACT pair -- and stalls the per-head DVE chain)
                    lnt = gp.tile([2, 512], f32, tag="lnt", bufs=2,
                                  name=f"lnt_{b}_{h}")
                    nc.scalar.activation(out=lnt, in_=csp[0:2, 0:512],
                                         func=Act.Ln)
                    rt = gp.tile([2, 512], f32, tag="rt", bufs=2,
                                 name=f"rt_{b}_{h}")
                    nc.scalar.activation(out=rt, in_=lnt, func=Act.Exp,
                                         scale=-1.0, bias=rbbc[:, :])
                    # broadcast r across partitions with a stride-0 DMA
                    # through a DRAM bounce (sync+gpsimd row halves)
                    nc.sync.dma_start(
                        out=rtd[b, h:h + 1, :].rearrange(
                            "a (c f) -> (a c) f", c=2),
                        in_=rt)
                    rb = dp.tile([128, HW], f32, tag="rb", bufs=2,
                                 name=f"rb_{b}_{h}")
                    nc.sync.dma_start(
                        out=rb[0:64, :],
                        in_=rtd[b, h:h + 1, :].to_broadcast([64, HW]))
                    nc.gpsimd.dma_start(
                        out=rb[64:128, :],
                        in_=rtd[b, h:h + 1, :].to_broadcast([64, HW]))
                    # normalize into the fp8 proj operand (16*o_norm), ch
                    # halves so proj's ch0 matmuls only wait on the first
                    for ch in range(2):
                        sl = slice(ch * 512, (ch + 1) * 512)
                        nc.vector.tensor_tensor(out=op[:, h % 2, sl],
                                                in0=ost[:, sl],
                                                in1=rb[:, sl], op=Alu.mult)

                # two-step software pipeline: the PE queue always holds a
                # full step of independent ST matmuls while ACT computes
                # the exp feeding the pending PV/cs.
                pend = []
                for h in range(NH):
                    for tp in range(NTP):
                        pend.append((h, tp, st_exp(h, tp)))
                        if len(pend) > PEND_DEPTH:
                            pv_cs(*pend.pop(0))
                    if mid_cb is not None and h in mid_cb:
                        mid_cb[h]()
                for p in pend:
                    pv_cs(*p)
                return o_pairs

            def proj(b, x_sb, o_pairs):
                # proj with DoubleRow over head pairs. Residual:
                #  - b=0 (overlapped with attention(1)): one DVE
                #    scalar_tensor_tensor per tile, x += psum/256.
                #  - b=1 (the kernel tail): accumulate 256*x INTO the PSUM
                #    with an f32r identity matmul, then evacuate on the
                #    (tail-idle) ACT engine per ch half -> DMA per half.
                #    Keeps the ~5us of serial DVE stt off the critical path.
                # (xr misses the gn-folded proj-bias, so the I-mm tail path
                # is only valid when proj_b is zero)
                tailb = TAIL_IMM and b == BLOC - 1 and not has_pbias
                for pair in ((0, 1), (2, 3)):
                    pus = {mt: ps.tile([128, 1024], f32, tag="st",
                                       name=f"pu_{b}_{mt}") for mt in pair}
                    for kp in range(2):
                        for mt in pair:
                            for ch in range(2):
                                nc.tensor.matmul(
                                    pus[mt][:, ch * 512:(ch + 1) * 512],
                                    lhsT=w_proj[:, 2 * kp:2 * kp + 2,
                                                mt * 128:(mt + 1) * 128],
                                    rhs=o_pairs[kp][:, :,
                                                    ch * 512:(ch + 1) * 512],
                                    start=(kp == 0),
                                    stop=(kp == 1 and not tailb),
                                    perf_mode=DR)
                    for mt in pair:
                        if tailb:
                            for ch in range(2):
                                sl = slice(ch * 512, (ch + 1) * 512)
                                nc.tensor.matmul(
                                    pus[mt][:, sl], lhsT=id256,
                                    rhs=xr_sb[:, mt, sl],
                                    start=False, stop=True)
                                nc.scalar.activation(
                                    out=x_sb[:, mt, sl], in_=pus[mt][:, sl],
                                    func=Act.Identity,
                                    scale=1.0 / (WSCALE * WSCALE))
                                eng = nc.sync if (mt + ch) % 2 == 0 else nc.gpsimd
                                eng.dma_start(
                                    out=out_d[b, mt * 128:(mt + 1) * 128, sl],
                                    in_=x_sb[:, mt, sl])
                        else:
                            nc.vector.scalar_tensor_tensor(
                                out=x_sb[:, mt, :], in0=pus[mt],
                                scalar=1.0 / (WSCALE * WSCALE),
                                in1=x_sb[:, mt, :],
                                op0=Alu.mult, op1=Alu.add)
                            eng = nc.sync if mt % 2 == 0 else nc.gpsimd
                            eng.dma_start(
                                out=out_d[b, mt * 128:(mt + 1) * 128, :],
                                in_=x_sb[:, mt, :])

            # ---------------- schedule --------------------------------------
            # gn(1) is emitted AFTER attention(0): its ACT squares would
            # otherwise sit in the in-order ACT queue (waiting on the x(1)
            # DMA) ahead of the q/k evacuations attention(0) needs.
            warmup("head", 18)
            s0 = gn_stats(0)
            xn0 = gn_finish(0, *s0)
            q0, k0, v0 = qkv(0, xn0)
            # batch 1's GN is emitted INSIDE attention(0) (stats after head
            # 1, finish after head 2): its DVE work fills attention's DVE
            # slack, and the ACT queue stays exp-only. Emitting it earlier
            # would stall attention(0)'s q/k evacuations on the x(1) DMA;
            # later would stall qkv(1) on the whole GN chain.
            s1, xn1b = {}, []
            if MID_GN:
                op0 = attention(0, q0, k0, v0, mid_cb={
                    0: lambda: gn_stats(1, on_dve=GN1_DVE, ts=[0, 1],
                                        store=s1),
                    1: lambda: gn_stats(1, on_dve=GN1_DVE, ts=[2, 3],
                                        store=s1),
                    2: lambda: xn1b.append(
                        gn_finish(1, s1["xn"], s1["pks"])),
                })
            else:
                op0 = attention(0, q0, k0, v0)
                xn1, pks1 = gn_stats(1, on_dve=GN1_DVE)
                xn1b.append(gn_finish(1, xn1, pks1))
            q1, k1, v1 = qkv(1, xn1b[0])
            proj(0, x_sbs[0], op0)
            op1 = attention(1, q1, k1, v1)
            proj(1, x_sbs[1], op1)

    nc.finalize()
    return nc


def kernel(x, gn_w, gn_b, qkv_w, qkv_b, proj_w, proj_b):
    import ml_dtypes

    from concourse.bass_utils import run_bass_kernel_spmd

    f8 = ml_dtypes.float8_e4m3
    qkv_b_arr = np.asarray(qkv_b, np.float32)
    has_vbias = bool(np.any(qkv_b_arr[2 * CH:3 * CH]))
    has_pbias = bool(np.any(np.asarray(proj_b, np.float32)))
    key = ("nc", has_vbias, has_pbias)
    if key not in _cache:
        _cache[key] = _build(has_vbias, has_pbias)
    nc = _cache[key]

    x = np.asarray(x, np.float32).reshape(B, CH, HW)
    qkv_w = np.asarray(qkv_w, np.float32)
    proj_w = np.asarray(proj_w, np.float32)
    qkv_b = qkv_b_arr
    shared = dict(
        wqkvT=np.ascontiguousarray(qkv_w.T * WSCALE).astype(f8),
        wprojT=np.ascontiguousarray(proj_w.T * WSCALE).astype(f8),
        gnw=np.ascontiguousarray(np.asarray(gn_w, np.float32).reshape(CT, 128).T),
        gnb=np.ascontiguousarray(np.asarray(gn_b, np.float32).reshape(CT, 128).T),
        qbqk=np.ascontiguousarray(qkv_b[0:2 * CH].reshape(2 * CT, 128).T),
        qbv=(qkv_b[2 * CH:3 * CH].reshape(1, CH) * WSCALE).astype(f8),
        pbcol=np.ascontiguousarray(np.asarray(proj_b, np.float32).reshape(CT, 128).T),
        **_consts(),
    )

    in_maps = []
    for c in range(NCORES):
        m = dict(shared)
        m["x"] = np.ascontiguousarray(x[c * BLOC:(c + 1) * BLOC])
        in_maps.append(m)

    kw = {}
    if TRACE:
        import shutil
        import axon_prof
        axon_prof.install()
        shutil.rmtree("/tmp/ktrace", ignore_errors=True)
        kw = dict(trace=True, tmpdir="/tmp/ktrace")
    res = run_bass_kernel_spmd(nc, in_maps, list(range(NCORES)), **kw)
    LAST["exec_time_ns"] = res.exec_time_ns
    LAST["trace"] = res.instructions_and_trace[1] if res.instructions_and_trace else None

    out = np.concatenate([res.results[c]["out"] for c in range(NCORES)], axis=0)
    return out.reshape(B, CH, 32, 32)
